# revision 3
# baseline (speedup 1.0000x reference)
"""Cross-attention decode kernel for Trainium2 (8 NeuronCores, Bass/Tile).

Reference computation (B=256, N=32768, D=1024, H=16, DH=64):
    qh = (q @ W_q.T)   [B,H,DH]
    kh = (k @ W_k.T)   [N,H,DH]
    vh = (v @ W_v.T)   [N,H,DH]
    score = einsum('bhd,nhd->hbn', qh, kh) / sqrt(DH)
    out   = einsum('hbn,nhd->bhd', softmax(score, -1), vh)  -> [B, D]

Sharding: split N across the 8 cores (flash-decoding style split-K).  Each
core projects its k/v shard, computes unnormalized exp-scores (no max
subtraction needed: scores ~ N(0,1), max < ~7, exp is safe in fp32), and
accumulates per-head numerator sum_n p*vh plus denominator sum_n p (the
denominator is obtained for free by appending a ones-column to vh in the
context matmul).  The host adds the 8 partial (num, den) pairs and divides.

Layout trick: every matmul contracts on the partition dim, so all operands
are staged pre-transposed from the host (kT, vT, W*T, qT).  Scores are
produced transposed [keys, b] so the context matmul needs no transposes
anywhere on the device.

Scores operands (khT, qh) are bf16, not f32r: an f32r matmul self-loads its
4-byte weights inside the MATMUL (no separate LDWEIGHTS), so the load can't
be pulled ahead — the first MM of every scores 4-group measured 313 ns
instead of ~110.  bf16 weights get a hideable FWL LDWEIGHTS.

Context matmul is b-major: lhsT=probs [keys,128b] (full 128-wide stationary,
FWL-eligible), rhs=vh [keys,65] so each MM streams only 65 columns instead
of 256 — the PE runs at full M=128 utilization.
"""

import sys

for _p in ("/opt/trn_rl_repo",):
    if _p not in sys.path:
        sys.path.insert(0, _p)

import numpy as np
import ml_dtypes

B, N, D, H = 256, 32768, 1024, 16
DH = D // H            # 64
NCORES = 8
NS = N // NCORES       # 4096 keys per core
SBK = 512              # keys per super-block
NSB = NS // SBK        # 8
KC = 128               # key chunk (scores/ctx granularity)
NKC = SBK // KC        # 4
DC = 128               # contraction chunk
NDC = D // DC          # 8
HG = 4                 # heads per scores-psum group
NHG = H // HG          # 4

_F16 = np.float16

_CACHED = {}


def _build():
    import concourse.mybir as mybir
    from concourse import bacc
    from concourse.tile import TileContext

    f16 = mybir.dt.float16
    f32 = mybir.dt.float32

    # Bacc (not raw Bass): its finalize() runs generate_event_semaphores,
    # which splits multi-sem waits into single-wait form (TRN2 ISA allows
    # one wait per instruction) — walrus rejects the IR otherwise.
    nc = bacc.Bacc()

    # host-swizzled layouts: qT/wkT/wvT are [128, c, ...] partition-major so
    # each DMA is fully contiguous per partition; wqT additionally has the
    # m-chunk outermost so the prologue can stream it in 8 small DMAs.
    qT = nc.declare_dram_parameter("qT", [128, NDC * B], f16, isOutput=False)
    wqT = nc.declare_dram_parameter("wqT", [NDC, 128, NDC * DC], f16, isOutput=False)
    wkT = nc.declare_dram_parameter("wkT", [128, NDC * D], f16, isOutput=False)
    wvT = nc.declare_dram_parameter("wvT", [128, NDC * D], f16, isOutput=False)
    kT = nc.declare_dram_parameter("kT", [D, NS], f16, isOutput=False)
    vT = nc.declare_dram_parameter("vT", [D, NS], f16, isOutput=False)
    # [b_lane, b_half, h, dh+1]: ctx numerator cols 0..DH-1, denominator col DH
    out = nc.declare_dram_parameter("out", [128, 2, H, DH + 1], f32, isOutput=True)

    Exp = mybir.ActivationFunctionType.Exp

    with TileContext(nc) as tc:
        with (
            tc.tile_pool(name="wk", bufs=1) as wk_pool,
            tc.tile_pool(name="wv", bufs=1) as wv_pool,
            tc.tile_pool(name="qh", bufs=1) as qh_pool,
            tc.tile_pool(name="cs", bufs=1) as cs_pool,
        ):
            # qh^T resident: [dout(part), dout_chunk, b] bf16
            qh_sb = qh_pool.tile([128, NDC, B], f16)
            # numerator/denominator accumulator: [b_lane, b_half, h, dh+1]
            ctx_sb = cs_pool.tile([128, 2, H, DH + 1], f32)
            nc.gpsimd.memset(ctx_sb, 0.0)

            # ---- prologue: qh^T = (q @ Wq.T)^T ----
            # Issue the q-side DMAs FIRST: HWDGE DMAs execute FIFO per
            # engine, so putting the big weight loads first would delay the
            # first matmul by ~20us.
            wk_sb = wk_pool.tile([128, NDC, D], f16)
            wv_sb = wv_pool.tile([128, NDC, D], f16)
            # qt/wq pools stay open for the whole kernel: releasing them lets
            # the kv pool reuse their SBUF range, which adds a false WAR dep
            # that stalls the first kt/vt DMAs ~10us behind the prologue.
            qt_pool = tc.alloc_tile_pool(name="qt", bufs=1)
            wq_pool = tc.alloc_tile_pool(name="wq", bufs=2)
            with (
                tc.tile_pool(name="pq", bufs=2, space="PSUM") as pq_pool,
            ):
                # PE warm-up: dummy matmuls during the initial DMA wait so the
                # HAM clock gate reaches 8/8 before the real pipeline starts.
                warm = qt_pool.tile([128, 512], f16, name="warm", tag="warm")
                nc.gpsimd.memset(warm, 0.0)
                wps = pq_pool.tile([128, 512], f32, name="wps", tag="wps")
                for _ in range(13):
                    nc.tensor.matmul(
                        wps, lhsT=warm[:, 0:128], rhs=warm[:, :],
                        start=True, stop=True,
                    )
                nc.vector.tensor_copy(out=warm[:, :], in_=wps)

                qt_sb = qt_pool.tile([128, NDC, B], f16)
                nc.sync.dma_start(out=qt_sb, in_=qT[:, :].rearrange("p (c b) -> p c b", c=NDC))
                wq_ts = []
                for m in range(NDC):
                    wq_t = wq_pool.tile([128, NDC, DC], f16, name="wq_t", bufs=NDC)
                    nc.sync.dma_start(out=wq_t, in_=wqT[m, :, :].rearrange("p (c n) -> p c n", c=NDC))
                    wq_ts.append(wq_t)
                # weight loads for the main loop, behind the prologue DMAs
                # (W_v is deferred into the sb==0 body so the first kt/vt
                # loads aren't queued behind it on the HWDGE FIFO)
                nc.sync.dma_start(out=wk_sb, in_=wkT[:, :].rearrange("p (c n) -> p c n", c=NDC))
                for m in range(NDC):
                    pq = pq_pool.tile([128, B], f32, name="pq")
                    for c in range(NDC):
                        nc.tensor.matmul(
                            pq,
                            lhsT=wq_ts[m][:, c, :],
                            rhs=qt_sb[:, c, :],
                            start=(c == 0),
                            stop=(c == NDC - 1),
                        )
                    nc.vector.tensor_copy(out=qh_sb[:, m, :], in_=pq)

            # ---- main loop over key super-blocks ----
            kT_v = kT[:, :].rearrange("(c p) n -> p c n", p=128)
            vT_v = vT[:, :].rearrange("(c p) n -> p c n", p=128)
            with (
                tc.tile_pool(name="kv", bufs=2) as kv_pool,
                tc.tile_pool(name="kh", bufs=2) as kh_pool,
                tc.tile_pool(name="vh", bufs=2) as vh_pool,
                tc.tile_pool(name="pr", bufs=14) as pr_pool,
                tc.tile_pool(name="pp", bufs=2, space="PSUM") as pp_pool,
                tc.tile_pool(name="ps", bufs=2, space="PSUM") as ps_pool,
                tc.tile_pool(name="pc", bufs=2, space="PSUM") as pc_pool,
            ):
                for sb in range(NSB):
                    ksl = slice(sb * SBK, (sb + 1) * SBK)
                    kt = kv_pool.tile([128, NDC, SBK], f16, tag="kt", name="kt", bufs=3)
                    for qtr in range(4):
                        cs = slice(qtr * NDC // 4, (qtr + 1) * NDC // 4)
                        nc.sync.dma_start(out=kt[:, cs, :], in_=kT_v[:, cs, ksl])
                    vt = kv_pool.tile([128, NDC, SBK], f16, tag="vt", name="vt")
                    for qtr in range(4):
                        cs = slice(qtr * NDC // 4, (qtr + 1) * NDC // 4)
                        nc.sync.dma_start(out=vt[:, cs, :], in_=vT_v[:, cs, ksl])
                    if sb == 0:
                        nc.sync.dma_start(
                            out=wv_sb,
                            in_=wvT[:, :].rearrange("p (c n) -> p c n", c=NDC),
                        )

                    # kh projection -> kh^T tile [dout(part), m_chunk, keys]
                    khT = kh_pool.tile([128, NDC, SBK], f16, name="khT")
                    for m in range(NDC):
                        pp = pp_pool.tile([128, SBK], f32, tag="pp", name="pp")
                        for c in range(NDC):
                            nc.tensor.matmul(
                                pp,
                                lhsT=wk_sb[:, c, m * DC:(m + 1) * DC],
                                rhs=kt[:, c, :],
                                start=(c == 0),
                                stop=(c == NDC - 1),
                            )
                        nc.vector.tensor_copy(out=khT[:, m, :], in_=pp)

                    # scores + exp: probs^T [keys(part), head, b] in bf16.
                    # Emission order alternates PE row group AND psum bank
                    # (even head -> bank0, odd head -> bank1, ...): row-tiled
                    # matmuls in different row groups execute concurrently on
                    # the PE, and alternating banks keeps every concurrent
                    # pair in different PSUM banks (same-bank pairs share a
                    # row group, which the array serializes) — a same-bank
                    # concurrent write is a fatal PSUM collision.
                    prmap = {}
                    for hg in range(NHG):
                        order = [
                            (hg * HG + 0, 0), (hg * HG + 1, 2),
                            (hg * HG + 2, 1), (hg * HG + 3, 3),
                        ]
                        for kcn in range(NKC):
                            ps = ps_pool.tile([128, HG, B], f32, name="ps")
                            for h, slot in order:
                                c = h // 2
                                off = (h % 2) * DH
                                nc.tensor.matmul(
                                    ps[:, slot, :],
                                    lhsT=khT[off:off + DH, c, kcn * KC:(kcn + 1) * KC],
                                    rhs=qh_sb[off:off + DH, c, :],
                                    start=True,
                                    stop=True,
                                    tile_position=(off, 0),
                                )
                            pr = pr_pool.tile([128, HG, B], f16, name="pr")
                            nc.scalar.activation(out=pr, in_=ps, func=Exp, scale=0.125)
                            for h, slot in order:
                                prmap[(h, kcn)] = (pr, slot)

                    # vh projection -> vh tile [keys(part), kcn, h, dh+1]
                    # (last column is 1.0 so the ctx matmul also yields the
                    #  softmax denominator on psum col DH).  Emitted after the
                    #  scores so the PE has work while W_v / vt arrive.
                    vh = vh_pool.tile([128, NKC, H, DH + 1], f16, name="vh")
                    nc.gpsimd.memset(vh[:, :, :, DH:DH + 1], 1.0)
                    for kcn in range(NKC):
                        for half in range(2):
                            pp2 = pp_pool.tile([128, SBK], f32, tag="pp", name="pp2")
                            for c in range(NDC):
                                nc.tensor.matmul(
                                    pp2,
                                    lhsT=vt[:, c, kcn * KC:(kcn + 1) * KC],
                                    rhs=wv_sb[:, c, half * 512:(half + 1) * 512],
                                    start=(c == 0),
                                    stop=(c == NDC - 1),
                                )
                            nc.vector.tensor_copy(
                                out=vh[:, kcn, half * 8:(half + 1) * 8, 0:DH],
                                in_=pp2.rearrange("p (h d) -> p h d", h=8),
                            )

                    # context accumulation, b-major: psum [b(128), dh+1] per
                    # (head, b-half); probs are the 128-wide stationary
                    # operand (FWL), vh streams only 65 columns.
                    for h in range(H):
                        for bh in range(2):
                            pc = pc_pool.tile([128, DH + 1], f32, name="pc")
                            for kcn in range(NKC):
                                pr, slot = prmap[(h, kcn)]
                                nc.tensor.matmul(
                                    pc,
                                    lhsT=pr[:, slot, bh * 128:(bh + 1) * 128],
                                    rhs=vh[:, kcn, h, :],
                                    start=(kcn == 0),
                                    stop=(kcn == NKC - 1),
                                )
                            nc.vector.tensor_add(
                                out=ctx_sb[:, bh, h, :],
                                in0=ctx_sb[:, bh, h, :],
                                in1=pc,
                            )
                        if sb == NSB - 1 and h % HG == HG - 1:
                            g0 = h - (HG - 1)
                            nc.sync.dma_start(
                                out=out[:, :, g0:h + 1, :],
                                in_=ctx_sb[:, :, g0:h + 1, :],
                            )

            wq_pool.release()
            qt_pool.release()

    # Run the bacc lowering passes (register allocation, wait splitting via
    # generate_event_semaphores, DCE).  The PJRT execution path serializes
    # nc.m as-is and never calls finalize, so this must happen here.
    nc.compile()
    return nc


def _get_nc():
    if "nc" not in _CACHED:
        _CACHED["nc"] = _build()
    return _CACHED["nc"]


def _swz(wT):
    """[D, X] -> [128, NDC*X] partition-major swizzle (c p) x -> p (c x)."""
    X = wT.shape[1]
    return np.ascontiguousarray(
        wT.reshape(NDC, 128, X).transpose(1, 0, 2).reshape(128, NDC * X)
    )


def _prep_inputs(q, k, v, W_q, W_k, W_v):
    """Host-side layout prep: transpose + cast to fp16, shard k/v by N."""
    qT = _swz(np.ascontiguousarray(q.T).astype(_F16))
    wqT_flat = np.ascontiguousarray(W_q.T).astype(_F16)
    # [c, p, m, j] -> [m, p, c, j]
    wqT = np.ascontiguousarray(
        wqT_flat.reshape(NDC, 128, NDC, DC).transpose(2, 1, 0, 3).reshape(NDC, 128, NDC * DC)
    )
    wkT = _swz(np.ascontiguousarray(W_k.T).astype(_F16))
    wvT = _swz(np.ascontiguousarray(W_v.T).astype(_F16))
    in_maps = []
    for core in range(NCORES):
        sl = slice(core * NS, (core + 1) * NS)
        in_maps.append(
            {
                "qT": qT,
                "wqT": wqT,
                "wkT": wkT,
                "wvT": wvT,
                "kT": np.ascontiguousarray(k[sl].T).astype(_F16),
                "vT": np.ascontiguousarray(v[sl].T).astype(_F16),
            }
        )
    return in_maps


def _combine(outs):
    """Sum per-core (num, den) partials and normalize: [128,2,16,65] x8 -> [B, D]."""
    S = np.zeros((128, 2, H, DH + 1), np.float32)
    for o in outs:
        S += np.asarray(o, np.float32)
    ctx = S[..., 0:DH] / S[..., DH:DH + 1]      # [128, 2, H, DH]
    # b = bh*128 + lane
    ctx = ctx.reshape(128, 2, H * DH).transpose(1, 0, 2).reshape(B, D)
    return np.ascontiguousarray(ctx).astype(np.float32)


def run(inputs, trace=False, trace_kwargs=None):
    from concourse.bass_utils import run_bass_kernel_spmd

    nc = _get_nc()
    in_maps = _prep_inputs(
        inputs["q"], inputs["k"], inputs["v"],
        inputs["W_q"], inputs["W_k"], inputs["W_v"],
    )
    res = run_bass_kernel_spmd(
        nc,
        in_maps,
        list(range(NCORES)),
        trace=trace,
        **(trace_kwargs or {}),
    )
    out = _combine([res.results[i]["out"] for i in range(NCORES)])
    return out, res


def kernel(**inputs):
    out, _ = run(inputs, trace=False)
    return out


# revision 5
# speedup vs baseline: 1.0063x; 1.0063x over previous
"""Cross-attention decode kernel for Trainium2 (8 NeuronCores, Bass/Tile).

Reference computation (B=256, N=32768, D=1024, H=16, DH=64):
    qh = (q @ W_q.T)   [B,H,DH]
    kh = (k @ W_k.T)   [N,H,DH]
    vh = (v @ W_v.T)   [N,H,DH]
    score = einsum('bhd,nhd->hbn', qh, kh) / sqrt(DH)
    out   = einsum('hbn,nhd->bhd', softmax(score, -1), vh)  -> [B, D]

Sharding: split N across the 8 cores (flash-decoding style split-K).  Each
core projects its k/v shard, computes unnormalized exp-scores (no max
subtraction needed: scores ~ N(0,1), max < ~7, exp is safe in fp32), and
accumulates per-head numerator sum_n p*vh plus denominator sum_n p (the
denominator is obtained for free by appending a ones-column to vh in the
context matmul).  The host adds the 8 partial (num, den) pairs and divides.

Layout trick: every matmul contracts on the partition dim, so all operands
are staged pre-transposed from the host (kT, vT, W*T, qT).  Scores are
produced transposed [keys, b] so the context matmul needs no transposes
anywhere on the device.

The whole datapath is fp16 (not bf16): same PE speed (1 col/cycle, FWL
LDWEIGHTS), but 3 extra mantissa bits keep the max-norm error ~5e-4 instead
of 1.9e-2 (the error is dominated by a few (b,h) pairs whose softmax is
concentrated; their top-score rounding error lands directly on the output).
f32r is avoided everywhere: an f32r matmul self-loads its 4-byte weights
inside the MATMUL, which can't be pulled ahead — it added ~200 ns to the
first MM of every scores group.

Context matmul is b-major: lhsT=probs [keys,128b] (full 128-wide stationary,
FWL-eligible), rhs=vh [keys,65] so each MM streams only 65 columns instead
of 256 — full M=128 PE utilization.

Emission interleaves the 16 scores groups 1:1 between projection/ctx psum
groups: every PSUM bank then has ~2.5 us before ring reuse, hiding the
~700 ns DVE CAST / ~1 us ACT exp drains that previously stalled the first
MM of the following group (~25 us of bank-wait stalls at bufs=2 spacing).
"""

import sys

for _p in ("/opt/trn_rl_repo",):
    if _p not in sys.path:
        sys.path.insert(0, _p)

import numpy as np

B, N, D, H = 256, 32768, 1024, 16
DH = D // H            # 64
NCORES = 8
NS = N // NCORES       # 4096 keys per core
SBK = 512              # keys per super-block
NSB = NS // SBK        # 8
KC = 128               # key chunk (scores/ctx granularity)
NKC = SBK // KC        # 4
DC = 128               # contraction chunk
NDC = D // DC          # 8
HG = 4                 # heads per scores-psum group
NHG = H // HG          # 4

_F16 = np.float16

_CACHED = {}


def _build():
    import concourse.mybir as mybir
    from concourse import bacc
    from concourse.tile import TileContext

    f16 = mybir.dt.float16
    f32 = mybir.dt.float32

    # Bacc (not raw Bass): its finalize() runs generate_event_semaphores,
    # which splits multi-sem waits into single-wait form (TRN2 ISA allows
    # one wait per instruction) — walrus rejects the IR otherwise.
    nc = bacc.Bacc()

    # host-swizzled layouts: qT/wkT/wvT are [128, c, ...] partition-major so
    # each DMA is fully contiguous per partition; wqT additionally has the
    # m-chunk outermost so the prologue can stream it in 8 small DMAs.
    qT = nc.declare_dram_parameter("qT", [128, NDC * B], f16, isOutput=False)
    wqT = nc.declare_dram_parameter("wqT", [NDC, 128, NDC * DC], f16, isOutput=False)
    wkT = nc.declare_dram_parameter("wkT", [128, NDC * D], f16, isOutput=False)
    wvT = nc.declare_dram_parameter("wvT", [128, NDC * D], f16, isOutput=False)
    kT = nc.declare_dram_parameter("kT", [D, NS], f16, isOutput=False)
    vT = nc.declare_dram_parameter("vT", [D, NS], f16, isOutput=False)
    # [b_lane, b_half, h, dh+1]: ctx numerator cols 0..DH-1, denominator col DH
    out = nc.declare_dram_parameter("out", [128, 2, H, DH + 1], f32, isOutput=True)

    Exp = mybir.ActivationFunctionType.Exp

    with TileContext(nc) as tc:
        with (
            tc.tile_pool(name="wk", bufs=1) as wk_pool,
            tc.tile_pool(name="wv", bufs=1) as wv_pool,
            tc.tile_pool(name="qh", bufs=1) as qh_pool,
            tc.tile_pool(name="cs", bufs=1) as cs_pool,
        ):
            # qh^T resident: [dout(part), dout_chunk, b]
            qh_sb = qh_pool.tile([128, NDC, B], f16)
            # numerator/denominator accumulator: [b_lane, b_half, h, dh+1]
            ctx_sb = cs_pool.tile([128, 2, H, DH + 1], f32)
            nc.gpsimd.memset(ctx_sb, 0.0)

            wk_sb = wk_pool.tile([128, NDC, D], f16)
            wv_sb = wv_pool.tile([128, NDC, D], f16)
            # qt/wq/kv pools stay open for the whole kernel: releasing qt/wq
            # lets a later pool reuse their SBUF range, which adds a false
            # WAR dep; kv is opened before the prologue so block-0 kt/vt
            # DMAs can be issued inside the prologue DMA sequence.
            qt_pool = tc.alloc_tile_pool(name="qt", bufs=1)
            wq_pool = tc.alloc_tile_pool(name="wq", bufs=2)
            kv_pool = tc.alloc_tile_pool(name="kv", bufs=2)

            kT_v = kT[:, :].rearrange("(c p) n -> p c n", p=128)
            vT_v = vT[:, :].rearrange("(c p) n -> p c n", p=128)
            wkT_v = wkT[:, :].rearrange("p (c n) -> p c n", c=NDC)

            def dma_kv(tag, src_v, ksl, bufs=2):
                t = kv_pool.tile([128, NDC, SBK], f16, tag=tag, name=tag, bufs=bufs)
                for qtr in range(4):
                    cs = slice(qtr * NDC // 4, (qtr + 1) * NDC // 4)
                    nc.sync.dma_start(out=t[:, cs, :], in_=src_v[:, cs, ksl])
                return t

            # ---- prologue: qh^T = (q @ Wq.T)^T ----
            # DMA issue order is the schedule: q-side first (q-projection is
            # the first PE work), then block-0 kt and the wk quarters in the
            # order the kh m-groups consume them, then vt0/wv for the vh
            # phase.  Everything is quartered so no single transfer gates.
            with (
                tc.tile_pool(name="pq", bufs=2, space="PSUM") as pq_pool,
            ):
                # PE warm-up: dummy matmuls during the initial DMA wait so the
                # HAM clock gate reaches 8/8 before the real pipeline starts.
                warm = qt_pool.tile([128, 512], f16, name="warm", tag="warm")
                nc.gpsimd.memset(warm, 0.0)
                wps = pq_pool.tile([128, 512], f32, name="wps", tag="wps")
                for _ in range(17):
                    nc.tensor.matmul(
                        wps, lhsT=warm[:, 0:128], rhs=warm[:, :],
                        start=True, stop=True,
                    )
                nc.vector.tensor_copy(out=warm[:, :], in_=wps)

                qt_sb = qt_pool.tile([128, NDC, B], f16)
                nc.sync.dma_start(out=qt_sb, in_=qT[:, :].rearrange("p (c b) -> p c b", c=NDC))
                wq_ts = []
                for m in range(NDC):
                    wq_t = wq_pool.tile([128, NDC, DC], f16, name="wq_t", bufs=NDC)
                    nc.sync.dma_start(out=wq_t, in_=wqT[m, :, :].rearrange("p (c n) -> p c n", c=NDC))
                    wq_ts.append(wq_t)
                kt0 = dma_kv("kt", kT_v, slice(0, SBK), bufs=3)
                for wq4 in range(4):
                    msl = slice(wq4 * 2 * DC, (wq4 + 1) * 2 * DC)
                    nc.sync.dma_start(out=wk_sb[:, :, msl], in_=wkT_v[:, :, msl])
                vt0 = dma_kv("vt", vT_v, slice(0, SBK))
                nc.sync.dma_start(
                    out=wv_sb,
                    in_=wvT[:, :].rearrange("p (c n) -> p c n", c=NDC),
                )
                for m in range(NDC):
                    pq = pq_pool.tile([128, B], f32, name="pq")
                    for c in range(NDC):
                        nc.tensor.matmul(
                            pq,
                            lhsT=wq_ts[m][:, c, :],
                            rhs=qt_sb[:, c, :],
                            start=(c == 0),
                            stop=(c == NDC - 1),
                        )
                    nc.vector.tensor_copy(out=qh_sb[:, m, :], in_=pq)

            # ---- main loop over key super-blocks ----
            with (
                tc.tile_pool(name="kh", bufs=2) as kh_pool,
                tc.tile_pool(name="vh", bufs=2) as vh_pool,
                tc.tile_pool(name="pr", bufs=18) as pr_pool,
                tc.tile_pool(name="pp", bufs=2, space="PSUM") as pp_pool,
                tc.tile_pool(name="ps", bufs=2, space="PSUM") as ps_pool,
                tc.tile_pool(name="pc", bufs=2, space="PSUM") as pc_pool,
            ):
                for sb in range(NSB):
                    if sb == 0:
                        kt, vt = kt0, vt0
                    else:
                        ksl = slice(sb * SBK, (sb + 1) * SBK)
                        kt = dma_kv("kt", kT_v, ksl, bufs=3)
                        vt = dma_kv("vt", vT_v, ksl)

                    khT = kh_pool.tile([128, NDC, SBK], f16, name="khT")
                    vh = vh_pool.tile([128, NKC, H, DH + 1], f16, name="vh")
                    nc.gpsimd.memset(vh[:, :, :, DH:DH + 1], 1.0)
                    prmap = {}

                    def kh_group(m):
                        pp = pp_pool.tile([128, SBK], f32, tag="pp", name="pp")
                        for c in range(NDC):
                            nc.tensor.matmul(
                                pp,
                                lhsT=wk_sb[:, c, m * DC:(m + 1) * DC],
                                rhs=kt[:, c, :],
                                start=(c == 0),
                                stop=(c == NDC - 1),
                            )
                        nc.vector.tensor_copy(out=khT[:, m, :], in_=pp)

                    def vh_group(g):
                        kcn, half = g // 2, g % 2
                        pp2 = pp_pool.tile([128, SBK], f32, tag="pp", name="pp2")
                        for c in range(NDC):
                            nc.tensor.matmul(
                                pp2,
                                lhsT=vt[:, c, kcn * KC:(kcn + 1) * KC],
                                rhs=wv_sb[:, c, half * 512:(half + 1) * 512],
                                start=(c == 0),
                                stop=(c == NDC - 1),
                            )
                        nc.vector.tensor_copy(
                            out=vh[:, kcn, half * 8:(half + 1) * 8, 0:DH],
                            in_=pp2.rearrange("p (h d) -> p h d", h=8),
                        )

                    # scores + exp for one (head-group, key-chunk):
                    # probs^T [keys(part), head, b] in fp16.  Emission order
                    # alternates PE row group AND psum bank: row-tiled
                    # matmuls in different row groups execute concurrently,
                    # and alternating banks keeps every concurrent pair in
                    # different PSUM banks (a same-bank concurrent write is
                    # a fatal PSUM collision).
                    def score_group(hg, kcn):
                        order = [
                            (hg * HG + 0, 0), (hg * HG + 1, 2),
                            (hg * HG + 2, 1), (hg * HG + 3, 3),
                        ]
                        ps = ps_pool.tile([128, HG, B], f32, name="ps")
                        for h, slot in order:
                            c = h // 2
                            off = (h % 2) * DH
                            nc.tensor.matmul(
                                ps[:, slot, :],
                                lhsT=khT[off:off + DH, c, kcn * KC:(kcn + 1) * KC],
                                rhs=qh_sb[off:off + DH, c, :],
                                start=True,
                                stop=True,
                                tile_position=(off, 0),
                            )
                        pr = pr_pool.tile([128, HG, B], f16, name="pr")
                        nc.scalar.activation(out=pr, in_=ps, func=Exp, scale=0.125)
                        for h, slot in order:
                            prmap[(h, kcn)] = (pr, slot)

                    # context accumulation, b-major: psum [b(128), dh+1] per
                    # (head, b-half); probs are the 128-wide stationary
                    # operand (FWL), vh streams only 65 columns.
                    def ctx_head(h):
                        for bh in range(2):
                            pc = pc_pool.tile([128, DH + 1], f32, name="pc")
                            for kcn in range(NKC):
                                pr, slot = prmap[(h, kcn)]
                                nc.tensor.matmul(
                                    pc,
                                    lhsT=pr[:, slot, bh * 128:(bh + 1) * 128],
                                    rhs=vh[:, kcn, h, :],
                                    start=(kcn == 0),
                                    stop=(kcn == NKC - 1),
                                )
                            nc.vector.tensor_add(
                                out=ctx_sb[:, bh, h, :],
                                in0=ctx_sb[:, bh, h, :],
                                in1=pc,
                            )
                        if sb == NSB - 1 and h % HG == HG - 1:
                            g0 = h - (HG - 1)
                            nc.sync.dma_start(
                                out=out[:, :, g0:h + 1, :],
                                in_=ctx_sb[:, :, g0:h + 1, :],
                            )

                    # Interleaved schedule: scores group (hg, kcn) needs khT
                    # chunks c = 2hg, 2hg+1, so hg0 can start after kh m1's
                    # CAST; spreading the 16 scores groups 1:1 between the
                    # psum-heavy proj/ctx groups gives every PSUM ring slot
                    # ~2.5 us of drain slack.
                    kh_group(0)
                    kh_group(1)
                    kh_group(2)
                    kh_group(3)
                    score_group(0, 0)
                    kh_group(4)
                    score_group(0, 1)
                    kh_group(5)
                    score_group(0, 2)
                    kh_group(6)
                    score_group(0, 3)
                    kh_group(7)
                    for g in range(8):          # vh groups, scores hg1/hg2
                        score_group(1 + g // 4, g % 4)
                        vh_group(g)
                    for j in range(4):          # scores hg3 between ctx heads
                        score_group(3, j)
                        ctx_head(j)
                    for h in range(4, H):
                        ctx_head(h)

            kv_pool.release()
            wq_pool.release()
            qt_pool.release()

    # Run the bacc lowering passes (register allocation, wait splitting via
    # generate_event_semaphores, DCE).  The PJRT execution path serializes
    # nc.m as-is and never calls finalize, so this must happen here.
    nc.compile()
    return nc


def _get_nc():
    if "nc" not in _CACHED:
        _CACHED["nc"] = _build()
    return _CACHED["nc"]


def _swz(wT):
    """[D, X] -> [128, NDC*X] partition-major swizzle (c p) x -> p (c x)."""
    X = wT.shape[1]
    return np.ascontiguousarray(
        wT.reshape(NDC, 128, X).transpose(1, 0, 2).reshape(128, NDC * X)
    )


def _prep_inputs(q, k, v, W_q, W_k, W_v):
    """Host-side layout prep: transpose + cast to fp16, shard k/v by N."""
    qT = _swz(np.ascontiguousarray(q.T).astype(_F16))
    wqT_flat = np.ascontiguousarray(W_q.T).astype(_F16)
    # [c, p, m, j] -> [m, p, c, j]
    wqT = np.ascontiguousarray(
        wqT_flat.reshape(NDC, 128, NDC, DC).transpose(2, 1, 0, 3).reshape(NDC, 128, NDC * DC)
    )
    wkT = _swz(np.ascontiguousarray(W_k.T).astype(_F16))
    wvT = _swz(np.ascontiguousarray(W_v.T).astype(_F16))
    in_maps = []
    for core in range(NCORES):
        sl = slice(core * NS, (core + 1) * NS)
        in_maps.append(
            {
                "qT": qT,
                "wqT": wqT,
                "wkT": wkT,
                "wvT": wvT,
                "kT": np.ascontiguousarray(k[sl].T).astype(_F16),
                "vT": np.ascontiguousarray(v[sl].T).astype(_F16),
            }
        )
    return in_maps


def _combine(outs):
    """Sum per-core (num, den) partials and normalize: [128,2,16,65] x8 -> [B, D]."""
    S = np.zeros((128, 2, H, DH + 1), np.float32)
    for o in outs:
        S += np.asarray(o, np.float32)
    ctx = S[..., 0:DH] / S[..., DH:DH + 1]      # [128, 2, H, DH]
    # b = bh*128 + lane
    ctx = ctx.reshape(128, 2, H * DH).transpose(1, 0, 2).reshape(B, D)
    return np.ascontiguousarray(ctx).astype(np.float32)


def run(inputs, trace=False, trace_kwargs=None):
    from concourse.bass_utils import run_bass_kernel_spmd

    nc = _get_nc()
    in_maps = _prep_inputs(
        inputs["q"], inputs["k"], inputs["v"],
        inputs["W_q"], inputs["W_k"], inputs["W_v"],
    )
    res = run_bass_kernel_spmd(
        nc,
        in_maps,
        list(range(NCORES)),
        trace=trace,
        **(trace_kwargs or {}),
    )
    out = _combine([res.results[i]["out"] for i in range(NCORES)])
    return out, res


def kernel(**inputs):
    out, _ = run(inputs, trace=False)
    return out


# revision 8
# speedup vs baseline: 1.0160x; 1.0096x over previous
"""Cross-attention decode kernel for Trainium2 (8 NeuronCores, Bass/Tile).

Reference computation (B=256, N=32768, D=1024, H=16, DH=64):
    qh = (q @ W_q.T)   [B,H,DH]
    kh = (k @ W_k.T)   [N,H,DH]
    vh = (v @ W_v.T)   [N,H,DH]
    score = einsum('bhd,nhd->hbn', qh, kh) / sqrt(DH)
    out   = einsum('hbn,nhd->bhd', softmax(score, -1), vh)  -> [B, D]

Sharding: split N across the 8 cores (flash-decoding style split-K).  Each
core projects its k/v shard, computes unnormalized exp-scores (no max
subtraction needed: scores ~ N(0,1), max < ~7, exp is safe in fp32), and
accumulates per-head numerator sum_n p*vh plus denominator sum_n p (the
denominator is obtained for free by appending a ones-column to vh in the
context matmul).  The host adds the 8 partial (num, den) pairs and divides.

Layout trick: every matmul contracts on the partition dim, so all operands
are staged pre-transposed from the host (kT, vT, W*T, qT).  Scores are
produced transposed [keys, b] so the context matmul needs no transposes
anywhere on the device.

The whole datapath is fp16 (not bf16): same PE speed (1 col/cycle, FWL
LDWEIGHTS), but 3 extra mantissa bits keep the max-norm error ~5e-4 instead
of 1.9e-2 (the error is dominated by a few (b,h) pairs whose softmax is
concentrated; their top-score rounding error lands directly on the output).
f32r is avoided everywhere: an f32r matmul self-loads its 4-byte weights
inside the MATMUL, which can't be pulled ahead — it added ~200 ns to the
first MM of every scores group.

Context matmul is b-major: lhsT=probs [keys,128b] (full 128-wide stationary,
FWL-eligible), rhs=vh [keys,65] so each MM streams only 65 columns instead
of 256 — full M=128 PE utilization.

Emission interleaves the 16 scores groups 1:1 between projection/ctx psum
groups: every PSUM bank then has ~2.5 us before ring reuse, hiding the
~700 ns DVE CAST / ~1 us ACT exp drains that previously stalled the first
MM of the following group (~25 us of bank-wait stalls at bufs=2 spacing).
"""

import sys

for _p in ("/opt/trn_rl_repo",):
    if _p not in sys.path:
        sys.path.insert(0, _p)

import numpy as np

B, N, D, H = 256, 32768, 1024, 16
DH = D // H            # 64
NCORES = 8
NS = N // NCORES       # 4096 keys per core
SBK = 512              # keys per super-block
NSB = NS // SBK        # 8
KC = 128               # key chunk (scores/ctx granularity)
NKC = SBK // KC        # 4
DC = 128               # contraction chunk
NDC = D // DC          # 8
HG = 4                 # heads per scores-psum group
NHG = H // HG          # 4

_F16 = np.float16

_CACHED = {}


def _build():
    import concourse.mybir as mybir
    from concourse import bacc
    from concourse.tile import TileContext

    f16 = mybir.dt.float16
    f32 = mybir.dt.float32

    # Bacc (not raw Bass): its finalize() runs generate_event_semaphores,
    # which splits multi-sem waits into single-wait form (TRN2 ISA allows
    # one wait per instruction) — walrus rejects the IR otherwise.
    nc = bacc.Bacc()

    # host-swizzled layouts: qT/wkT/wvT are [128, c, ...] partition-major so
    # each DMA is fully contiguous per partition; wqT additionally has the
    # m-chunk outermost so the prologue can stream it in 8 small DMAs.
    qT = nc.declare_dram_parameter("qT", [128, NDC * B], f16, isOutput=False)
    wqT = nc.declare_dram_parameter("wqT", [NDC, 128, NDC * DC], f16, isOutput=False)
    wkT = nc.declare_dram_parameter("wkT", [128, NDC * D], f16, isOutput=False)
    wvT = nc.declare_dram_parameter("wvT", [128, NDC * D], f16, isOutput=False)
    kT = nc.declare_dram_parameter("kT", [D, NS], f16, isOutput=False)
    vT = nc.declare_dram_parameter("vT", [D, NS], f16, isOutput=False)
    # [b_lane, b_half, h, dh+1]: ctx numerator cols 0..DH-1, denominator col DH
    out = nc.declare_dram_parameter("out", [128, 2, H, DH + 1], f32, isOutput=True)

    Exp = mybir.ActivationFunctionType.Exp

    with TileContext(nc) as tc:
        with (
            tc.tile_pool(name="wk", bufs=1) as wk_pool,
            tc.tile_pool(name="wv", bufs=1) as wv_pool,
            tc.tile_pool(name="qh", bufs=1) as qh_pool,
            tc.tile_pool(name="cs", bufs=1) as cs_pool,
        ):
            # qh^T resident: [dout(part), dout_chunk, b]
            qh_sb = qh_pool.tile([128, NDC, B], f16)
            # numerator/denominator accumulator: [b_lane, b_half, h, dh+1]
            ctx_sb = cs_pool.tile([128, 2, H, DH + 1], f32)
            nc.gpsimd.memset(ctx_sb, 0.0)

            wk_sb = wk_pool.tile([128, NDC, D], f16)
            wv_sb = wv_pool.tile([128, NDC, D], f16)
            # qt/wq/kv pools stay open for the whole kernel: releasing qt/wq
            # lets a later pool reuse their SBUF range, which adds a false
            # WAR dep; kv is opened before the prologue so block-0 kt/vt
            # DMAs can be issued inside the prologue DMA sequence.
            qt_pool = tc.alloc_tile_pool(name="qt", bufs=1)
            wq_pool = tc.alloc_tile_pool(name="wq", bufs=2)
            kv_pool = tc.alloc_tile_pool(name="kv", bufs=2)

            kT_v = kT[:, :].rearrange("(c p) n -> p c n", p=128)
            vT_v = vT[:, :].rearrange("(c p) n -> p c n", p=128)
            wkT_v = wkT[:, :].rearrange("p (c n) -> p c n", c=NDC)

            def dma_kv(tag, src_v, ksl, bufs=2):
                t = kv_pool.tile([128, NDC, SBK], f16, tag=tag, name=tag, bufs=bufs)
                for qtr in range(4):
                    cs = slice(qtr * NDC // 4, (qtr + 1) * NDC // 4)
                    nc.sync.dma_start(out=t[:, cs, :], in_=src_v[:, cs, ksl])
                return t

            # ---- prologue: qh^T = (q @ Wq.T)^T ----
            # DMA issue order is the schedule: q-side first (q-projection is
            # the first PE work), then block-0 kt and the wk quarters in the
            # order the kh m-groups consume them, then vt0/wv for the vh
            # phase.  Everything is quartered so no single transfer gates.
            with (
                tc.tile_pool(name="pq", bufs=2, space="PSUM") as pq_pool,
            ):
                # PE warm-up: dummy matmuls during the initial DMA wait so the
                # HAM clock gate reaches 8/8 before the real pipeline starts.
                warm = qt_pool.tile([128, 512], f16, name="warm", tag="warm")
                nc.gpsimd.memset(warm, 0.0)
                wps = pq_pool.tile([128, 512], f32, name="wps", tag="wps")
                for _ in range(17):
                    nc.tensor.matmul(
                        wps, lhsT=warm[:, 0:128], rhs=warm[:, :],
                        start=True, stop=True,
                    )
                nc.vector.tensor_copy(out=warm[:, :], in_=wps)

                qt_sb = qt_pool.tile([128, NDC, B], f16)
                nc.sync.dma_start(out=qt_sb, in_=qT[:, :].rearrange("p (c b) -> p c b", c=NDC))
                wq_ts = []
                for m in range(NDC):
                    wq_t = wq_pool.tile([128, NDC, DC], f16, name="wq_t", bufs=NDC)
                    nc.sync.dma_start(out=wq_t, in_=wqT[m, :, :].rearrange("p (c n) -> p c n", c=NDC))
                    wq_ts.append(wq_t)
                kt0 = dma_kv("kt", kT_v, slice(0, SBK), bufs=3)
                for wq4 in range(4):
                    msl = slice(wq4 * 2 * DC, (wq4 + 1) * 2 * DC)
                    nc.sync.dma_start(out=wk_sb[:, :, msl], in_=wkT_v[:, :, msl])
                vt0 = dma_kv("vt", vT_v, slice(0, SBK))
                nc.sync.dma_start(
                    out=wv_sb,
                    in_=wvT[:, :].rearrange("p (c n) -> p c n", c=NDC),
                )
                for m in range(NDC):
                    pq = pq_pool.tile([128, B], f32, name="pq")
                    for c in range(NDC):
                        nc.tensor.matmul(
                            pq,
                            lhsT=wq_ts[m][:, c, :],
                            rhs=qt_sb[:, c, :],
                            start=(c == 0),
                            stop=(c == NDC - 1),
                        )
                    nc.vector.tensor_copy(out=qh_sb[:, m, :], in_=pq)

            # ---- main loop over key super-blocks ----
            with (
                tc.tile_pool(name="kh", bufs=2) as kh_pool,
                tc.tile_pool(name="vh", bufs=2) as vh_pool,
                tc.tile_pool(name="pr", bufs=18) as pr_pool,
                tc.tile_pool(name="pp", bufs=2, space="PSUM") as pp_pool,
                tc.tile_pool(name="ps", bufs=2, space="PSUM") as ps_pool,
                tc.tile_pool(name="pc", bufs=2, space="PSUM") as pc_pool,
            ):
                for sb in range(NSB):
                    if sb == 0:
                        kt, vt = kt0, vt0
                    else:
                        ksl = slice(sb * SBK, (sb + 1) * SBK)
                        kt = dma_kv("kt", kT_v, ksl, bufs=3)
                        vt = dma_kv("vt", vT_v, ksl)

                    khT = kh_pool.tile([128, NDC, SBK], f16, name="khT")
                    vh = vh_pool.tile([128, NKC, H, DH + 1], f16, name="vh")
                    nc.gpsimd.memset(vh[:, :, :, DH:DH + 1], 1.0)
                    prmap = {}

                    def kh_group(m):
                        pp = pp_pool.tile([128, SBK], f32, tag="pp", name="pp")
                        for c in range(NDC):
                            nc.tensor.matmul(
                                pp,
                                lhsT=wk_sb[:, c, m * DC:(m + 1) * DC],
                                rhs=kt[:, c, :],
                                start=(c == 0),
                                stop=(c == NDC - 1),
                            )
                        nc.vector.tensor_copy(out=khT[:, m, :], in_=pp)

                    def vh_group(g):
                        half, kcn = g // 4, g % 4
                        pp2 = pp_pool.tile([128, SBK], f32, tag="pp", name="pp2")
                        for c in range(NDC):
                            nc.tensor.matmul(
                                pp2,
                                lhsT=vt[:, c, kcn * KC:(kcn + 1) * KC],
                                rhs=wv_sb[:, c, half * 512:(half + 1) * 512],
                                start=(c == 0),
                                stop=(c == NDC - 1),
                            )
                        nc.vector.tensor_copy(
                            out=vh[:, kcn, half * 8:(half + 1) * 8, 0:DH],
                            in_=pp2.rearrange("p (h d) -> p h d", h=8),
                        )

                    # scores + exp for one (head-group, key-chunk):
                    # probs^T [keys(part), head, b] in fp16.  Emission order
                    # alternates PE row group AND psum bank: row-tiled
                    # matmuls in different row groups execute concurrently,
                    # and alternating banks keeps every concurrent pair in
                    # different PSUM banks (a same-bank concurrent write is
                    # a fatal PSUM collision).
                    def score_group(hg, kcn):
                        order = [
                            (hg * HG + 0, 0), (hg * HG + 1, 2),
                            (hg * HG + 2, 1), (hg * HG + 3, 3),
                        ]
                        ps = ps_pool.tile([128, HG, B], f32, name="ps")
                        for h, slot in order:
                            c = h // 2
                            off = (h % 2) * DH
                            nc.tensor.matmul(
                                ps[:, slot, :],
                                lhsT=khT[off:off + DH, c, kcn * KC:(kcn + 1) * KC],
                                rhs=qh_sb[off:off + DH, c, :],
                                start=True,
                                stop=True,
                                tile_position=(off, 0),
                            )
                        pr = pr_pool.tile([128, HG, B], f16, name="pr")
                        # two bank-granular activations: slots 0,1 sit in the
                        # tile's first PSUM bank, 2,3 in the second; splitting
                        # the exp releases each bank ~500 ns earlier for the
                        # next scores group (ps ring is only 2 deep).
                        nc.scalar.activation(
                            out=pr[:, 0:2, :], in_=ps[:, 0:2, :], func=Exp, scale=0.125
                        )
                        nc.scalar.activation(
                            out=pr[:, 2:4, :], in_=ps[:, 2:4, :], func=Exp, scale=0.125
                        )
                        for h, slot in order:
                            prmap[(h, kcn)] = (pr, slot)

                    # context accumulation, b-major: psum [b(128), dh+1] per
                    # (head, b-half); probs are the 128-wide stationary
                    # operand (FWL), vh streams only 65 columns.
                    def ctx_head(h):
                        for bh in range(2):
                            pc = pc_pool.tile([128, DH + 1], f32, name="pc")
                            for kcn in range(NKC):
                                pr, slot = prmap[(h, kcn)]
                                nc.tensor.matmul(
                                    pc,
                                    lhsT=pr[:, slot, bh * 128:(bh + 1) * 128],
                                    rhs=vh[:, kcn, h, :],
                                    start=(kcn == 0),
                                    stop=(kcn == NKC - 1),
                                )
                            nc.vector.tensor_add(
                                out=ctx_sb[:, bh, h, :],
                                in0=ctx_sb[:, bh, h, :],
                                in1=pc,
                            )
                        if sb == NSB - 1 and h % HG == HG - 1:
                            g0 = h - (HG - 1)
                            nc.sync.dma_start(
                                out=out[:, :, g0:h + 1, :],
                                in_=ctx_sb[:, :, g0:h + 1, :],
                            )

                    # Interleaved schedule: scores group (hg, kcn) needs khT
                    # chunks c = 2hg, 2hg+1, so hg0 can start after kh m1's
                    # CAST; spreading the 16 scores groups 1:1 between the
                    # psum-heavy proj/ctx groups gives every PSUM ring slot
                    # ~2.5 us of drain slack.
                    kh_group(0)
                    kh_group(1)
                    kh_group(2)
                    kh_group(3)
                    score_group(0, 0)
                    kh_group(4)
                    score_group(0, 1)
                    kh_group(5)
                    score_group(0, 2)
                    kh_group(6)
                    score_group(0, 3)
                    kh_group(7)
                    for g in range(4):          # vh half-0 groups, scores hg1
                        score_group(1, g)
                        vh_group(g)
                    for g in range(4):          # ctx h0-3 + vh half-1, scores hg2
                        score_group(2, g)
                        ctx_head(g)
                        vh_group(4 + g)
                    for j in range(4):          # scores hg3 between ctx heads
                        score_group(3, j)
                        ctx_head(4 + j)
                    for h in range(8, H):
                        ctx_head(h)

            kv_pool.release()
            wq_pool.release()
            qt_pool.release()

    # Run the bacc lowering passes (register allocation, wait splitting via
    # generate_event_semaphores, DCE).  The PJRT execution path serializes
    # nc.m as-is and never calls finalize, so this must happen here.
    nc.compile()
    return nc


def _get_nc():
    if "nc" not in _CACHED:
        _CACHED["nc"] = _build()
    return _CACHED["nc"]


def _swz(wT):
    """[D, X] -> [128, NDC*X] partition-major swizzle (c p) x -> p (c x)."""
    X = wT.shape[1]
    return np.ascontiguousarray(
        wT.reshape(NDC, 128, X).transpose(1, 0, 2).reshape(128, NDC * X)
    )


def _prep_inputs(q, k, v, W_q, W_k, W_v):
    """Host-side layout prep: transpose + cast to fp16, shard k/v by N."""
    qT = _swz(np.ascontiguousarray(q.T).astype(_F16))
    wqT_flat = np.ascontiguousarray(W_q.T).astype(_F16)
    # [c, p, m, j] -> [m, p, c, j]
    wqT = np.ascontiguousarray(
        wqT_flat.reshape(NDC, 128, NDC, DC).transpose(2, 1, 0, 3).reshape(NDC, 128, NDC * DC)
    )
    wkT = _swz(np.ascontiguousarray(W_k.T).astype(_F16))
    wvT = _swz(np.ascontiguousarray(W_v.T).astype(_F16))
    in_maps = []
    for core in range(NCORES):
        sl = slice(core * NS, (core + 1) * NS)
        in_maps.append(
            {
                "qT": qT,
                "wqT": wqT,
                "wkT": wkT,
                "wvT": wvT,
                "kT": np.ascontiguousarray(k[sl].T).astype(_F16),
                "vT": np.ascontiguousarray(v[sl].T).astype(_F16),
            }
        )
    return in_maps


def _combine(outs):
    """Sum per-core (num, den) partials and normalize: [128,2,16,65] x8 -> [B, D]."""
    S = np.zeros((128, 2, H, DH + 1), np.float32)
    for o in outs:
        S += np.asarray(o, np.float32)
    ctx = S[..., 0:DH] / S[..., DH:DH + 1]      # [128, 2, H, DH]
    # b = bh*128 + lane
    ctx = ctx.reshape(128, 2, H * DH).transpose(1, 0, 2).reshape(B, D)
    return np.ascontiguousarray(ctx).astype(np.float32)


def run(inputs, trace=False, trace_kwargs=None):
    from concourse.bass_utils import run_bass_kernel_spmd

    nc = _get_nc()
    in_maps = _prep_inputs(
        inputs["q"], inputs["k"], inputs["v"],
        inputs["W_q"], inputs["W_k"], inputs["W_v"],
    )
    res = run_bass_kernel_spmd(
        nc,
        in_maps,
        list(range(NCORES)),
        trace=trace,
        **(trace_kwargs or {}),
    )
    out = _combine([res.results[i]["out"] for i in range(NCORES)])
    return out, res


def kernel(**inputs):
    out, _ = run(inputs, trace=False)
    return out


# revision 11
# speedup vs baseline: 1.0254x; 1.0093x over previous
"""Cross-attention decode kernel for Trainium2 (8 NeuronCores, Bass/Tile).

Reference computation (B=256, N=32768, D=1024, H=16, DH=64):
    qh = (q @ W_q.T)   [B,H,DH]
    kh = (k @ W_k.T)   [N,H,DH]
    vh = (v @ W_v.T)   [N,H,DH]
    score = einsum('bhd,nhd->hbn', qh, kh) / sqrt(DH)
    out   = einsum('hbn,nhd->bhd', softmax(score, -1), vh)  -> [B, D]

Sharding: split N across the 8 cores (flash-decoding style split-K).  Each
core projects its k/v shard, computes unnormalized exp-scores (no max
subtraction needed: scores ~ N(0,1), max < ~7, exp is safe in fp32), and
accumulates per-head numerator sum_n p*vh plus denominator sum_n p (the
denominator is obtained for free by appending a ones-column to vh in the
context matmul).  The host adds the 8 partial (num, den) pairs and divides.

Layout trick: every matmul contracts on the partition dim, so all operands
are staged pre-transposed from the host (kT, vT, W*T, qT).  Scores are
produced transposed [keys, b] so the context matmul needs no transposes
anywhere on the device.

The whole datapath is fp16 (not bf16): same PE speed (1 col/cycle, FWL
LDWEIGHTS), but 3 extra mantissa bits keep the max-norm error ~5e-4 instead
of 1.9e-2 (the error is dominated by a few (b,h) pairs whose softmax is
concentrated; their top-score rounding error lands directly on the output).
f32r is avoided everywhere: an f32r matmul self-loads its 4-byte weights
inside the MATMUL, which can't be pulled ahead — it added ~200 ns to the
first MM of every scores group.

Context matmul is b-major: lhsT=probs [keys,128b] (full 128-wide stationary,
FWL-eligible), rhs=vh [keys,65] so each MM streams only 65 columns instead
of 256 — full M=128 PE utilization.

Emission interleaves the 16 scores groups 1:1 between projection/ctx psum
groups: every PSUM bank then has ~2.5 us before ring reuse, hiding the
~700 ns DVE CAST / ~1 us ACT exp drains that previously stalled the first
MM of the following group (~25 us of bank-wait stalls at bufs=2 spacing).
"""

import sys

for _p in ("/opt/trn_rl_repo",):
    if _p not in sys.path:
        sys.path.insert(0, _p)

import numpy as np

B, N, D, H = 256, 32768, 1024, 16
DH = D // H            # 64
NCORES = 8
NS = N // NCORES       # 4096 keys per core
SBK = 512              # keys per super-block
NSB = NS // SBK        # 8
KC = 128               # key chunk (scores/ctx granularity)
NKC = SBK // KC        # 4
DC = 128               # contraction chunk
NDC = D // DC          # 8
HG = 4                 # heads per scores-psum group
NHG = H // HG          # 4

_F16 = np.float16

_CACHED = {}


def _build():
    import concourse.mybir as mybir
    from concourse import bacc
    from concourse.tile import TileContext

    f16 = mybir.dt.float16
    f32 = mybir.dt.float32

    # Bacc (not raw Bass): its finalize() runs generate_event_semaphores,
    # which splits multi-sem waits into single-wait form (TRN2 ISA allows
    # one wait per instruction) — walrus rejects the IR otherwise.
    nc = bacc.Bacc()

    # host-swizzled layouts: qT/wkT/wvT are [128, c, ...] partition-major so
    # each DMA is fully contiguous per partition; wqT additionally has the
    # m-chunk outermost so the prologue can stream it in 8 small DMAs.
    qT = nc.declare_dram_parameter("qT", [128, NDC * B], f16, isOutput=False)
    wqT = nc.declare_dram_parameter("wqT", [NDC, 128, NDC * DC], f16, isOutput=False)
    wkT = nc.declare_dram_parameter("wkT", [128, NDC * D], f16, isOutput=False)
    wvT = nc.declare_dram_parameter("wvT", [128, NDC * D], f16, isOutput=False)
    kT = nc.declare_dram_parameter("kT", [D, NS], f16, isOutput=False)
    vT = nc.declare_dram_parameter("vT", [D, NS], f16, isOutput=False)
    # [b_lane, b_half, h, dh+1]: ctx numerator cols 0..DH-1, denominator col DH
    out = nc.declare_dram_parameter("out", [128, 2, H, DH + 1], f32, isOutput=True)

    Exp = mybir.ActivationFunctionType.Exp

    with TileContext(nc) as tc:
        with (
            tc.tile_pool(name="wk", bufs=1) as wk_pool,
            tc.tile_pool(name="wv", bufs=1) as wv_pool,
            tc.tile_pool(name="qh", bufs=1) as qh_pool,
            tc.tile_pool(name="cs", bufs=1) as cs_pool,
        ):
            # qh^T resident: [dout(part), dout_chunk, b]
            qh_sb = qh_pool.tile([128, NDC, B], f16)
            # numerator/denominator accumulator: [b_lane, b_half, h, dh+1]
            ctx_sb = cs_pool.tile([128, 2, H, DH + 1], f32)
            nc.gpsimd.memset(ctx_sb, 0.0)

            wk_sb = wk_pool.tile([128, NDC, D], f16)
            wv_sb = wv_pool.tile([128, NDC, D], f16)
            # qt/wq/kv pools stay open for the whole kernel: releasing qt/wq
            # lets a later pool reuse their SBUF range, which adds a false
            # WAR dep; kv is opened before the prologue so block-0 kt/vt
            # DMAs can be issued inside the prologue DMA sequence.
            qt_pool = tc.alloc_tile_pool(name="qt", bufs=1)
            wq_pool = tc.alloc_tile_pool(name="wq", bufs=2)
            kv_pool = tc.alloc_tile_pool(name="kv", bufs=2)

            kT_v = kT[:, :].rearrange("(c p) n -> p c n", p=128)
            vT_v = vT[:, :].rearrange("(c p) n -> p c n", p=128)
            wkT_v = wkT[:, :].rearrange("p (c n) -> p c n", c=NDC)

            def dma_kv(tag, src_v, ksl, bufs=2):
                t = kv_pool.tile([128, NDC, SBK], f16, tag=tag, name=tag, bufs=bufs)
                for qtr in range(4):
                    cs = slice(qtr * NDC // 4, (qtr + 1) * NDC // 4)
                    nc.sync.dma_start(out=t[:, cs, :], in_=src_v[:, cs, ksl])
                return t

            # ---- prologue: qh^T = (q @ Wq.T)^T ----
            # DMA issue order is the schedule: q-side first (q-projection is
            # the first PE work), then block-0 kt and the wk quarters in the
            # order the kh m-groups consume them, then vt0/wv for the vh
            # phase.  Everything is quartered so no single transfer gates.
            with (
                tc.tile_pool(name="pq", bufs=2, space="PSUM") as pq_pool,
            ):
                # PE warm-up: dummy matmuls during the initial DMA wait so the
                # HAM clock gate reaches 8/8 before the real pipeline starts.
                warm = qt_pool.tile([128, 512], f16, name="warm", tag="warm")
                nc.gpsimd.memset(warm, 0.0)
                wps = pq_pool.tile([128, 512], f32, name="wps", tag="wps")
                for _ in range(17):
                    nc.tensor.matmul(
                        wps, lhsT=warm[:, 0:128], rhs=warm[:, :],
                        start=True, stop=True,
                    )
                nc.vector.tensor_copy(out=warm[:, :], in_=wps)

                qt_sb = qt_pool.tile([128, NDC, B], f16)
                nc.sync.dma_start(out=qt_sb, in_=qT[:, :].rearrange("p (c b) -> p c b", c=NDC))
                wq_ts = []
                for m in range(NDC):
                    wq_t = wq_pool.tile([128, NDC, DC], f16, name="wq_t", bufs=NDC)
                    nc.sync.dma_start(out=wq_t, in_=wqT[m, :, :].rearrange("p (c n) -> p c n", c=NDC))
                    wq_ts.append(wq_t)
                kt0 = dma_kv("kt", kT_v, slice(0, SBK), bufs=3)
                for wq4 in range(4):
                    msl = slice(wq4 * 2 * DC, (wq4 + 1) * 2 * DC)
                    nc.sync.dma_start(out=wk_sb[:, :, msl], in_=wkT_v[:, :, msl])
                vt0 = dma_kv("vt", vT_v, slice(0, SBK))
                nc.sync.dma_start(
                    out=wv_sb,
                    in_=wvT[:, :].rearrange("p (c n) -> p c n", c=NDC),
                )
                for m in range(NDC):
                    pq = pq_pool.tile([128, B], f32, name="pq")
                    for c in range(NDC):
                        nc.tensor.matmul(
                            pq,
                            lhsT=wq_ts[m][:, c, :],
                            rhs=qt_sb[:, c, :],
                            start=(c == 0),
                            stop=(c == NDC - 1),
                        )
                    nc.vector.tensor_copy(out=qh_sb[:, m, :], in_=pq)

            # ---- main loop over key super-blocks ----
            with (
                tc.tile_pool(name="kh", bufs=2) as kh_pool,
                tc.tile_pool(name="vh", bufs=2) as vh_pool,
                tc.tile_pool(name="pr", bufs=18) as pr_pool,
                tc.tile_pool(name="pp", bufs=2, space="PSUM") as pp_pool,
                tc.tile_pool(name="ps", bufs=2, space="PSUM") as ps_pool,
                tc.tile_pool(name="pc", bufs=2, space="PSUM") as pc_pool,
            ):
                for sb in range(NSB):
                    if sb == 0:
                        kt, vt = kt0, vt0
                    else:
                        ksl = slice(sb * SBK, (sb + 1) * SBK)
                        kt = dma_kv("kt", kT_v, ksl, bufs=3)
                        vt = dma_kv("vt", vT_v, ksl)

                    khT = kh_pool.tile([128, NDC, SBK], f16, name="khT")
                    vh = vh_pool.tile([128, NKC, H, DH + 1], f16, name="vh")
                    nc.gpsimd.memset(vh[:, :, :, DH:DH + 1], 1.0)
                    prmap = {}

                    def kh_group(m):
                        pp = pp_pool.tile([128, SBK], f32, tag="pp", name="pp")
                        for c in range(NDC):
                            nc.tensor.matmul(
                                pp,
                                lhsT=wk_sb[:, c, m * DC:(m + 1) * DC],
                                rhs=kt[:, c, :],
                                start=(c == 0),
                                stop=(c == NDC - 1),
                            )
                        nc.vector.tensor_copy(out=khT[:, m, :], in_=pp)

                    def vh_group(g):
                        half, kcn = g // 4, g % 4
                        pp2 = pp_pool.tile([128, SBK], f32, tag="pp", name="pp2")
                        for c in range(NDC):
                            nc.tensor.matmul(
                                pp2,
                                lhsT=vt[:, c, kcn * KC:(kcn + 1) * KC],
                                rhs=wv_sb[:, c, half * 512:(half + 1) * 512],
                                start=(c == 0),
                                stop=(c == NDC - 1),
                            )
                        nc.vector.tensor_copy(
                            out=vh[:, kcn, half * 8:(half + 1) * 8, 0:DH],
                            in_=pp2.rearrange("p (h d) -> p h d", h=8),
                        )

                    # scores + exp for one (head-group, key-chunk):
                    # probs^T [keys(part), head, b] in fp16.  Emission order
                    # alternates PE row group AND psum bank: row-tiled
                    # matmuls in different row groups execute concurrently,
                    # and alternating banks keeps every concurrent pair in
                    # different PSUM banks (a same-bank concurrent write is
                    # a fatal PSUM collision).
                    def score_group(hg, kcn):
                        order = [
                            (hg * HG + 0, 0), (hg * HG + 1, 2),
                            (hg * HG + 2, 1), (hg * HG + 3, 3),
                        ]
                        ps = ps_pool.tile([128, HG, B], f32, name="ps")
                        for h, slot in order:
                            c = h // 2
                            off = (h % 2) * DH
                            nc.tensor.matmul(
                                ps[:, slot, :],
                                lhsT=khT[off:off + DH, c, kcn * KC:(kcn + 1) * KC],
                                rhs=qh_sb[off:off + DH, c, :],
                                start=True,
                                stop=True,
                                tile_position=(off, 0),
                            )
                        pr = pr_pool.tile([128, HG, B], f16, name="pr")
                        nc.scalar.activation(out=pr, in_=ps, func=Exp, scale=0.125)
                        for h, slot in order:
                            prmap[(h, kcn)] = (pr, slot)

                    # context accumulation, b-major: psum [b(128), 2, dh+1]
                    # holds a PAIR of heads per bank per b-half; probs are
                    # the 128-wide stationary operand (FWL), vh streams only
                    # 65 columns.  Pairing heads halves the DVE add count
                    # (the adds were stalling the pc ring).
                    def ctx_head2(h0):
                        for bh in range(2):
                            pc = pc_pool.tile([128, 2, DH + 1], f32, name="pc")
                            for j in range(2):
                                h = h0 + j
                                for kcn in range(NKC):
                                    pr, slot = prmap[(h, kcn)]
                                    nc.tensor.matmul(
                                        pc[:, j, :],
                                        lhsT=pr[:, slot, bh * 128:(bh + 1) * 128],
                                        rhs=vh[:, kcn, h, :],
                                        start=(kcn == 0),
                                        stop=(kcn == NKC - 1),
                                    )
                            nc.vector.tensor_add(
                                out=ctx_sb[:, bh, h0:h0 + 2, :],
                                in0=ctx_sb[:, bh, h0:h0 + 2, :],
                                in1=pc,
                            )
                        if sb == NSB - 1 and (h0 + 2) % HG == 0:
                            g0 = h0 + 2 - HG
                            nc.sync.dma_start(
                                out=out[:, :, g0:h0 + 2, :],
                                in_=ctx_sb[:, :, g0:h0 + 2, :],
                            )

                    # Interleaved schedule: scores group (hg, kcn) needs khT
                    # chunks c = 2hg, 2hg+1, so hg0 can start after kh m1's
                    # CAST; spreading the 16 scores groups 1:1 between the
                    # psum-heavy proj/ctx groups gives every PSUM ring slot
                    # ~2.5 us of drain slack.
                    kh_group(0)
                    kh_group(1)
                    kh_group(2)
                    kh_group(3)
                    score_group(0, 0)
                    kh_group(4)
                    score_group(0, 1)
                    kh_group(5)
                    score_group(0, 2)
                    kh_group(6)
                    score_group(0, 3)
                    kh_group(7)
                    for g in range(4):          # vh half-0 groups, scores hg1
                        score_group(1, g)
                        vh_group(g)
                    for g in range(4):          # ctx h0-3 + vh half-1, scores hg2
                        score_group(2, g)
                        if g % 2 == 0:
                            ctx_head2(g)
                        vh_group(4 + g)
                    for j in range(4):          # scores hg3 between ctx pairs
                        score_group(3, j)
                        ctx_head2(4 + 2 * j)
                    for h0 in range(12, H, 2):
                        ctx_head2(h0)

            kv_pool.release()
            wq_pool.release()
            qt_pool.release()

    # Run the bacc lowering passes (register allocation, wait splitting via
    # generate_event_semaphores, DCE).  The PJRT execution path serializes
    # nc.m as-is and never calls finalize, so this must happen here.
    nc.compile()
    return nc


def _get_nc():
    if "nc" not in _CACHED:
        _CACHED["nc"] = _build()
    return _CACHED["nc"]


def _swz(wT):
    """[D, X] -> [128, NDC*X] partition-major swizzle (c p) x -> p (c x)."""
    X = wT.shape[1]
    return np.ascontiguousarray(
        wT.reshape(NDC, 128, X).transpose(1, 0, 2).reshape(128, NDC * X)
    )


def _prep_inputs(q, k, v, W_q, W_k, W_v):
    """Host-side layout prep: transpose + cast to fp16, shard k/v by N."""
    qT = _swz(np.ascontiguousarray(q.T).astype(_F16))
    wqT_flat = np.ascontiguousarray(W_q.T).astype(_F16)
    # [c, p, m, j] -> [m, p, c, j]
    wqT = np.ascontiguousarray(
        wqT_flat.reshape(NDC, 128, NDC, DC).transpose(2, 1, 0, 3).reshape(NDC, 128, NDC * DC)
    )
    wkT = _swz(np.ascontiguousarray(W_k.T).astype(_F16))
    wvT = _swz(np.ascontiguousarray(W_v.T).astype(_F16))
    in_maps = []
    for core in range(NCORES):
        sl = slice(core * NS, (core + 1) * NS)
        in_maps.append(
            {
                "qT": qT,
                "wqT": wqT,
                "wkT": wkT,
                "wvT": wvT,
                "kT": np.ascontiguousarray(k[sl].T).astype(_F16),
                "vT": np.ascontiguousarray(v[sl].T).astype(_F16),
            }
        )
    return in_maps


def _combine(outs):
    """Sum per-core (num, den) partials and normalize: [128,2,16,65] x8 -> [B, D]."""
    S = np.zeros((128, 2, H, DH + 1), np.float32)
    for o in outs:
        S += np.asarray(o, np.float32)
    ctx = S[..., 0:DH] / S[..., DH:DH + 1]      # [128, 2, H, DH]
    # b = bh*128 + lane
    ctx = ctx.reshape(128, 2, H * DH).transpose(1, 0, 2).reshape(B, D)
    return np.ascontiguousarray(ctx).astype(np.float32)


def run(inputs, trace=False, trace_kwargs=None):
    from concourse.bass_utils import run_bass_kernel_spmd

    nc = _get_nc()
    in_maps = _prep_inputs(
        inputs["q"], inputs["k"], inputs["v"],
        inputs["W_q"], inputs["W_k"], inputs["W_v"],
    )
    res = run_bass_kernel_spmd(
        nc,
        in_maps,
        list(range(NCORES)),
        trace=trace,
        **(trace_kwargs or {}),
    )
    out = _combine([res.results[i]["out"] for i in range(NCORES)])
    return out, res


def kernel(**inputs):
    out, _ = run(inputs, trace=False)
    return out
